# revision 40
# baseline (speedup 1.0000x reference)
"""TRN2 Bass kernel for nn_AttentionCell (BitLinear GQA attention cell).

Sharding (8 cores): data-parallel over batch (2) x tensor-parallel over the
4 KV head-groups (4 query heads each). Each core computes Q/K/V projections,
causal softmax attention for its 4 heads, and a row-parallel partial of the
output projection; the host sums the 4 partials per batch and applies the
final RMSNorm row scale.

Precision: x streams as an fp16 hi/lo pair (~22-bit); Q and K are fp16
pairs ([ql;qh]/[kh;kl]); scores use a 2-pass matmul (one 128-row cross
pass, one 65-row aug pass [kh;ones] x [qh;-rowmax]). The ternary weights
are exactly representable in fp16. The row max comes from a separate fp16
stats pass in [row,key] layout (masked diag block, DVE row-reduce),
negated+transposed back via a padded matmul against -I (columns at
stride 32 so the rows land on legal partition offsets). exp on ACT to
fp16 P; causal zeroing of the diagonal P block runs on GPSIMD post-exp;
V is single fp16 in [token,feat] layout with a ones column that yields
softmax denominators inside the PV matmul; normalization multiplies by a
GPSIMD partition-broadcast of the reciprocal denominator row.

Scheduling: the row-max stats for row-chunk 0 are software-pipelined into
the projection phase; in the attention phase a flat (row-chunk, head)
pipeline overlaps each step's PE work with the stats reduces of the step
three ahead (emission order: augrow(i+1), attn(i), stats(i+3)), keeping
DVE (reduces) and PE (matmuls) concurrently busy.
"""

import numpy as np
import ml_dtypes

import concourse.bass as bass
import concourse.bacc as bacc
import concourse.mybir as mybir
import concourse.tile as tile
from concourse.bass_utils import run_bass_kernel_spmd
from concourse.masks import make_identity

f32 = mybir.dt.float32
f16 = mybir.dt.float16

EPS = np.float32(1.1920929e-07)
B, T, D = 2, 2048, 1024
H, HKV, HD = 16, 4, 64
NH = 4            # local (per-core) query heads
LF = NH * HD      # 256 local q features
P = 128
DT = D // P       # 8 d-tiles
CH = 4            # 512-wide token chunks
CW = 512
QB = T // P       # 16 query row blocks
KB = T // P       # 16 key blocks
NEG = -1.0e30

Exp = mybir.ActivationFunctionType.Exp
AOp = mybir.AluOpType
AxX = mybir.AxisListType.X

# 1 = x in single fp16 (q/k pairs absorb projection rounding only);
# 2 = x as fp16 hi/lo pair (22-bit x, bulletproof precision)
X_PASSES = 2


def _build():
    nc = bacc.Bacc("TRN2", target_bir_lowering=False, debug=False)

    xh_d = nc.dram_tensor("xh", [D, T], f16, kind="ExternalInput").ap()
    if X_PASSES == 2:
        xl_d = nc.dram_tensor("xl", [D, T], f16, kind="ExternalInput").ap()
    std_d = nc.dram_tensor("stdc", [P, KB], f32, kind="ExternalInput").ap()
    wq = nc.dram_tensor("wq", [D, LF], f16, kind="ExternalInput").ap()
    wkv = nc.dram_tensor("wkv", [D, P], f16, kind="ExternalInput").ap()
    wo = nc.dram_tensor("wo", [LF, D], f16, kind="ExternalInput").ap()
    yp = nc.dram_tensor("yp", [T, D], f32, kind="ExternalOutput").ap()
    ssqa = nc.dram_tensor("ssqa", [1, T], f32, kind="ExternalOutput").ap()

    with tile.TileContext(nc) as tc:
        with (
            tc.tile_pool(name="const", bufs=1) as const,
            tc.tile_pool(name="persist", bufs=1) as persist,
        ):
            ident16 = const.tile([P, P], f16, tag="ident16")
            make_identity(nc, ident16[:])
            negident = const.tile([P, P], f16, tag="negident")
            make_identity(nc, negident[:])
            nc.gpsimd.tensor_scalar_mul(negident[:], negident[:], -1.0)
            # stats mask, positioned in the LAST 128 cols of a 512 window
            # (S layout [row, key]): key > row -> NEG
            mask512 = const.tile([P, CW], f32, tag="mask512")
            nc.gpsimd.memset(mask512[:], 0.0)
            nc.gpsimd.affine_select(
                out=mask512[:, CW - P:CW], in_=mask512[:, CW - P:CW],
                compare_op=AOp.is_ge, fill=NEG,
                base=0, pattern=[[-1, P]], channel_multiplier=1,
            )

            stdc = persist.tile([P, KB], f32, tag="stdc")
            nc.sync.dma_start(out=stdc[:], in_=std_d[:])
            wq_sb = persist.tile([P, DT, LF], f16, tag="wq_sb")
            nc.sync.dma_start(
                out=wq_sb[:], in_=wq.rearrange("(dt p) f -> p dt f", p=P)
            )
            wkv_sb = persist.tile([P, DT, P], f16, tag="wkv_sb")
            nc.sync.dma_start(
                out=wkv_sb[:], in_=wkv.rearrange("(dt p) f -> p dt f", p=P)
            )
            wo_sb = persist.tile([P, 2, D], f16, tag="wo_sb")
            nc.sync.dma_start(
                out=wo_sb[:], in_=wo.rearrange("(ft p) o -> p ft o", p=P)
            )

            # QTaug = [qh; -max], QTs = [ql; qh]; KTaug = [kh; ones],
            # KTs = [kh; kl].  The lo parts absorb the psum->fp16 rounding
            # (and, with X_PASSES == 2, the x rounding as well).
            QTaug = [persist.tile([65, T], f16, tag=f"qta{h}", name=f"qta{h}")
                     for h in range(NH)]
            QTs = [persist.tile([P, T], f16, tag=f"qts{h}", name=f"qts{h}")
                   for h in range(NH)]
            KTaug = persist.tile([65, T], f16, tag="kta")
            KTs = persist.tile([P, T], f16, tag="kts")
            nc.gpsimd.memset(KTaug[64:65, :], 1.0)
            # V in transposed [token, feat] layout, fp16, col 64 = ones
            Vsb = [persist.tile([P, 65], f16, tag=f"v{kb}", name=f"v{kb}")
                   for kb in range(KB)]
            for kb in range(KB):
                nc.gpsimd.memset(Vsb[kb][:, 64:65], 1.0)
            aTh = [persist.tile([P, T], f16, tag=f"ath{ft}", name=f"ath{ft}")
                   for ft in range(2)]
            onesc = persist.tile([P, 1], f16, tag="onesc")
            nc.gpsimd.memset(onesc[:], 1.0)
            ssqrow = persist.tile([1, T], f32, tag="ssqrow")

            # --- stats helpers: software-pipelined masked row maxes ---
            # mallA holds per-head max columns at stride 32 so the
            # transposed rows land on legal partition offsets (0/32/64/96)
            def emit_stats(mstat, ps_stat, rc, h, mallA_h):
                nc.gpsimd.memset(mallA_h[:], 0.0)
                for j in range(4):
                    qb = rc * 4 + j
                    nk = (qb + 1) * P
                    nchunks = (nk + CW - 1) // CW
                    mxs = mstat.tile([P, 4], f32, tag="mxs")
                    for ic in range(nchunks):
                        k0 = ic * CW
                        kw = min(CW, nk - k0)
                        last = ic == nchunks - 1
                        pS = ps_stat.tile([P, CW], f32, tag="pstat")
                        nc.tensor.matmul(
                            pS[:, :kw],
                            QTaug[h][0:HD, qb * P:(qb + 1) * P],
                            KTaug[0:HD, k0:k0 + kw],
                            start=True, stop=True,
                        )
                        if last:
                            # diag-block mask add, then row max
                            nc.vector.tensor_add(
                                pS[:, kw - P:kw], pS[:, kw - P:kw],
                                mask512[:, CW - P:CW])
                            dst = (mallA_h[:, 32 * j:32 * j + 1]
                                   if nchunks == 1 else mxs[:, ic:ic + 1])
                            nc.vector.reduce_max(dst, pS[:, :kw], axis=AxX)
                        else:
                            nc.vector.reduce_max(mxs[:, ic:ic + 1],
                                                 pS[:, :kw], axis=AxX)
                    if nchunks > 1:
                        nc.vector.reduce_max(mallA_h[:, 32 * j:32 * j + 1],
                                             mxs[:, 0:nchunks], axis=AxX)

            def emit_augrow(ps_nm, rc, h, mallA_h):
                # negated transpose of the row maxes for one head
                pnm = ps_nm.tile([97, P], f32, tag="pnm",
                                 name=f"pnm_{rc}_{h}")
                nc.tensor.matmul(pnm[:], mallA_h[:], negident[:],
                                 start=True, stop=True)
                for j in range(4):
                    qb = rc * 4 + j
                    row = pnm[32 * j:32 * j + 1, :]
                    dst = QTaug[h][HD:HD + 1, qb * P:(qb + 1) * P]
                    nc.vector.tensor_copy(dst, row)

            with tc.tile_pool(name="mstat", bufs=2) as mstat:
                mallA = {}

                def alloc_mallA(rc, h):
                    mallA[(rc, h)] = mstat.tile(
                        [P, 97], f16, tag=f"malla{h}",
                        name=f"malla_{rc}_{h}", bufs=3)
                    return mallA[(rc, h)]

                # -------------- phase 2: projections + stats(0) --------------
                with (
                    tc.tile_pool(name="ph2", bufs=8) as ph2,
                    tc.tile_pool(name="ph2v", bufs=2) as ph2v,
                    tc.tile_pool(name="ps_q", bufs=3, space="PSUM") as ps_q,
                    tc.tile_pool(name="ps_kv", bufs=2, space="PSUM") as ps_kv,
                    tc.tile_pool(name="ps_vt", bufs=1, space="PSUM") as ps_vt,
                    tc.tile_pool(name="ps_st0", bufs=2, space="PSUM") as ps_st0,
                ):
                    for c in range(CH):
                        cs = slice(c * CW, (c + 1) * CW)
                        psQ = [ps_q.tile([P, CW], f32, tag="psq",
                                         name=f"psq_{c}_{i}") for i in range(2)]
                        psKV = ps_kv.tile([P, CW], f32, tag="pskv")
                        for d in range(DT):
                            th = ph2.tile([P, CW], f16, tag="th")
                            nc.sync.dma_start(out=th[:],
                                              in_=xh_d[d * P:(d + 1) * P, cs])
                            xparts = [th]
                            if X_PASSES == 2:
                                tl = ph2.tile([P, CW], f16, tag="tl")
                                nc.sync.dma_start(
                                    out=tl[:], in_=xl_d[d * P:(d + 1) * P, cs])
                                xparts.append(tl)
                            first, last = d == 0, d == DT - 1
                            for ft in range(2):
                                for xi, tx in enumerate(xparts):
                                    nc.tensor.matmul(
                                        psQ[ft][:],
                                        wq_sb[:, d, ft * P:(ft + 1) * P], tx[:],
                                        start=first and xi == 0,
                                        stop=last and xi == len(xparts) - 1)
                            for xi, tx in enumerate(xparts):
                                nc.tensor.matmul(
                                    psKV[:], wkv_sb[:, d, :], tx[:],
                                    start=first and xi == 0,
                                    stop=last and xi == len(xparts) - 1)
                        for ft in range(2):
                            for sub in range(2):
                                h = 2 * ft + sub
                                pslc = psQ[ft][sub * HD:(sub + 1) * HD, :]
                                nc.scalar.copy(QTaug[h][0:HD, cs], pslc)
                                nc.gpsimd.tensor_copy(QTs[h][HD:P, cs],
                                                      QTaug[h][0:HD, cs])
                                nc.vector.tensor_sub(QTs[h][0:HD, cs], pslc,
                                                     QTaug[h][0:HD, cs])
                        nc.scalar.copy(KTaug[0:HD, cs], psKV[0:HD, :])
                        nc.gpsimd.tensor_copy(KTs[0:HD, cs], KTaug[0:HD, cs])
                        nc.vector.tensor_sub(KTs[HD:P, cs], psKV[0:HD, :],
                                             KTaug[0:HD, cs])
                        # V (rows 64:128 of psKV) -> fp16, transpose to
                        # [token, feat], un-normalize by per-token std
                        vt = ph2v.tile([HD, CW], f16, tag="vt")
                        nc.vector.tensor_copy(vt[:], psKV[HD:P, :])
                        for s4 in range(4):
                            kb = c * 4 + s4
                            pvt = ps_vt.tile([P, HD], f16, tag="pvt")
                            nc.tensor.transpose(pvt[:],
                                                vt[:, s4 * P:(s4 + 1) * P],
                                                ident16[0:HD, 0:HD])
                            nc.scalar.mul(Vsb[kb][:, 0:HD], pvt[:],
                                          stdc[:, kb:kb + 1])
                        # stats(rc=0, head c) rides along with chunk c
                        emit_stats(mstat, ps_st0, 0, c, alloc_mallA(0, c))

                # ------ phases 3-6: attention + O-proj, per row-chunk ------
                with (
                    tc.tile_pool(name="ph3", bufs=6) as ph3,
                    tc.tile_pool(name="ptp", bufs=1) as ptp,
                    tc.tile_pool(name="ph6", bufs=4) as ph6,
                    tc.tile_pool(name="ps_stat", bufs=2, space="PSUM") as ps_stat,
                    tc.tile_pool(name="ps_st", bufs=2, space="PSUM") as ps_st,
                    tc.tile_pool(name="ps_nm", bufs=1, space="PSUM") as ps_nm,
                    tc.tile_pool(name="ps_y", bufs=2, space="PSUM") as ps_y,
                ):
                    PTt = [ptp.tile([P, CW], f16, tag=f"pt{kc}", name=f"pt{kc}")
                           for kc in range(KB)]
                    # flat (rc, h) pipeline with one-step stats lookahead:
                    # attn(step i) overlaps stats+augrow of step i+1, whose
                    # DVE size tracks step i's PE size
                    steps = [(rc, h) for rc in range(4) for h in range(NH)]
                    emit_augrow(ps_nm, 0, 0, mallA[(0, 0)])
                    for i, (rc, h) in enumerate(steps):
                        rs = slice(rc * CW, (rc + 1) * CW)
                        # augrow for the NEXT step first so its small copies
                        # sit early in the DVE/ACT queues
                        if i + 1 < len(steps):
                            rcn, hn = steps[i + 1]
                            emit_augrow(ps_nm, rcn, hn, mallA[(rcn, hn)])
                        attention_head(nc, tc, ph3, ptp, ph6, ps_st, ps_y,
                                       PTt, QTaug, QTs, KTaug, KTs, Vsb,
                                       aTh, rc, h, rs)
                        # three-step stats lookahead
                        if i == 0:
                            for ii in (2, 3):
                                rc2, h2 = steps[ii]
                                if rc2 >= 1:
                                    emit_stats(mstat, ps_stat, rc2, h2,
                                               alloc_mallA(rc2, h2))
                        elif i + 3 < len(steps):
                            rc2, h2 = steps[i + 3]
                            if rc2 >= 1:
                                emit_stats(mstat, ps_stat, rc2, h2,
                                           alloc_mallA(rc2, h2))
                        if h == NH - 1:
                            # ssq of the normalized attn rows + O-projection
                            psq = ps_y.tile([1, CW], f32, tag="psy",
                                            name=f"psq_{rc}")
                            for ft in range(2):
                                sqt = ph6.tile([P, CW], f16, tag="sqt")
                                nc.gpsimd.tensor_mul(sqt[:], aTh[ft][:, rs],
                                                     aTh[ft][:, rs])
                                nc.tensor.matmul(psq[:], onesc[:], sqt[:],
                                                 start=(ft == 0),
                                                 stop=(ft == 1))
                            nc.vector.tensor_copy(ssqrow[0:1, rs], psq[:])
                            for j in range(4):
                                qb = rc * 4 + j
                                qs = slice(qb * P, (qb + 1) * P)
                                for oc in range(2):
                                    os_ = slice(oc * CW, (oc + 1) * CW)
                                    psY = ps_y.tile([P, CW], f32, tag="psy",
                                                    name=f"psY_{qb}_{oc}")
                                    for ft in range(2):
                                        nc.tensor.matmul(
                                            psY[:], aTh[ft][:, qs],
                                            wo_sb[:, ft, os_],
                                            start=(ft == 0), stop=(ft == 1))
                                    ysb = ph6.tile([P, CW], f32, tag="ysb")
                                    if rc == 3:
                                        nc.vector.tensor_copy(ysb[:], psY[:])
                                    else:
                                        nc.scalar.copy(ysb[:], psY[:])
                                    nc.sync.dma_start(out=yp[qs, os_],
                                                      in_=ysb[:])
                    nc.sync.dma_start(out=ssqa[:], in_=ssqrow[:])

    nc.finalize()
    return nc


def attention_head(nc, tc, ph3, ptp, ph6, ps_st, ps_y, PTt, QTaug, QTs,
                   KTaug, KTs, Vsb, aTh, rc, h, rs):
    # S.T for this row chunk: 2-pass (cross terms via [kh;kl]x[ql;qh],
    # hi terms + -max via the 65-row aug pass) + exp; diagonal P block
    # zeroed on GPSIMD post-exp
    for kc in range((rc + 1) * 4):
        c0 = max(kc * P, rc * CW)
        cw_ = (rc + 1) * CW - c0
        kslc = slice(kc * P, (kc + 1) * P)
        cslc = slice(c0, c0 + cw_)
        lo = c0 - rc * CW
        pST = ps_st.tile([P, CW], f32, tag="pst")
        nc.tensor.matmul(pST[:, lo:lo + cw_], KTs[:, kslc],
                         QTs[h][:, cslc], start=True, stop=False)
        nc.tensor.matmul(pST[:, lo:lo + cw_], KTaug[:, kslc],
                         QTaug[h][:, cslc], start=False, stop=True)
        nc.scalar.activation(PTt[kc][:, lo:lo + cw_],
                             pST[:, lo:lo + cw_], Exp)
        if c0 == kc * P:
            # zero P above the diagonal: keep row >= key
            nc.gpsimd.affine_select(
                out=PTt[kc][:, lo:lo + P], in_=PTt[kc][:, lo:lo + P],
                compare_op=AOp.is_ge, fill=0.0,
                base=0, pattern=[[1, P]], channel_multiplier=-1,
            )

    # transposed PV: psO rows = [PV | denom]
    psO = ps_st.tile([65, CW], f32, tag="pso", bufs=1)
    for kc in range((rc + 1) * 4):
        c0 = max(kc * P, rc * CW)
        cw_ = (rc + 1) * CW - c0
        lo = c0 - rc * CW
        nc.tensor.matmul(
            psO[:, lo:lo + cw_], Vsb[kc][:, 0:65], PTt[kc][:, lo:lo + cw_],
            start=(kc == 0), stop=(kc == (rc + 1) * 4 - 1),
        )
    # normalize columns by 1/denom via gpsimd broadcast
    rd32 = ph3.tile([1, CW], f32, tag="rd32")
    nc.vector.reciprocal(rd32[:], psO[64:65, :])
    dbcS = ph3.tile([HD, CW], f32, tag="dbcs")
    nc.gpsimd.partition_broadcast(dbcS[:], rd32[:], channels=HD)
    ft, sub = h // 2, h % 2
    nc.vector.tensor_mul(aTh[ft][sub * HD:(sub + 1) * HD, rs],
                         psO[0:HD, :], dbcS[:])


def _ternary(w):
    th = np.abs(w).mean(dtype=np.float64)
    return (np.sign(w) * (np.abs(w) > th)).astype(np.float32)


_CACHE = {}


def kernel(x, q_w, q_g, k_w, k_g, v_w, o_w, o_g, qk_gain):
    x = np.asarray(x, np.float32)
    wq_eff = (_ternary(np.asarray(q_w)) * np.asarray(q_g)[None, :]
              * np.float32(qk_gain) / np.float32(np.sqrt(np.float32(HD))))
    wk_eff = _ternary(np.asarray(k_w)) * np.asarray(k_g)[None, :]
    wo_eff = _ternary(np.asarray(o_w)) * np.asarray(o_g)[None, :]
    wqT = np.ascontiguousarray(wq_eff.T).astype(np.float16)      # [D, H*HD]
    wkT = wk_eff.T.astype(np.float16)                            # [D, HKV*HD]
    wvT = np.asarray(v_w, np.float32).T.astype(np.float16)
    woT = np.ascontiguousarray(wo_eff.T).astype(np.float16)      # [D, D]

    # per-token rms scales (host); x_hat = x * r, V un-normalized on device
    # by std = 1/r
    xs = x.astype(np.float64)
    ssq = (xs * xs).mean(-1) + np.float64(EPS)
    r = (1.0 / np.sqrt(ssq)).astype(np.float32)                  # [B, T]
    std = np.sqrt(ssq).astype(np.float32)                        # [B, T]
    xn = (x * r[:, :, None]).astype(np.float32)

    if "nc" not in _CACHE:
        _CACHE["nc"] = _build()
    nc = _CACHE["nc"]

    in_maps = []
    for core in range(8):
        b, g = divmod(core, 4)
        xnT = np.ascontiguousarray(xn[b].T)                      # [D, T] f32
        xh = xnT.astype(np.float16)
        wkv_c = np.concatenate(
            [wkT[:, g * HD:(g + 1) * HD], wvT[:, g * HD:(g + 1) * HD]], axis=1)
        im = {
            "xh": xh,
            "stdc": np.ascontiguousarray(std[b].reshape(KB, P).T),
            "wq": np.ascontiguousarray(wqT[:, g * LF:(g + 1) * LF]),
            "wkv": np.ascontiguousarray(wkv_c),
            "wo": np.ascontiguousarray(woT[g * LF:(g + 1) * LF, :]),
        }
        if X_PASSES == 2:
            im["xl"] = (xnT - xh.astype(np.float32)).astype(np.float16)
        in_maps.append(im)
    _CACHE["in_maps"] = in_maps
    res = run_bass_kernel_spmd(nc, in_maps, list(range(8)))

    out = np.empty((B, T, D), np.float32)
    for b in range(B):
        ssq_a = np.zeros((T,), np.float32)
        ysum = np.zeros((T, D), np.float32)
        for g in range(4):
            rr = res.results[b * 4 + g]
            ysum += rr["yp"]
            ssq_a += rr["ssqa"][0]
        ro = 1.0 / np.sqrt(ssq_a / np.float32(D) + EPS)
        out[b] = ysum * ro[:, None]
    return out


if __name__ == "__main__":
    data = np.load("/root/problem/inputs.npz")
    out = kernel(**{k: data[k] for k in data.files})
    ref = np.load("/root/problem/ref_out.npy")
    d = out.astype(np.float64) - ref.astype(np.float64)
    rv = (d * d).sum() / (ref.astype(np.float64) ** 2).sum()
    print("resid_var=%.3e relerr=%.3e absmax=%.3g" %
          (rv, np.sqrt(rv), np.abs(d).max()))


# revision 41
# speedup vs baseline: 1.0801x; 1.0801x over previous
"""TRN2 Bass kernel for nn_AttentionCell (BitLinear GQA attention cell).

Sharding (8 cores): data-parallel over batch (2) x tensor-parallel over the
4 KV head-groups (4 query heads each). Each core computes Q/K/V projections,
causal softmax attention for its 4 heads, and a row-parallel partial of the
output projection; the host sums the 4 partials per batch and applies the
final RMSNorm row scale.

Precision: x streams as an fp16 hi/lo pair (~22-bit); Q and K are fp16
pairs ([ql;qh]/[kh;kl]); scores use a 2-pass matmul (one 128-row cross
pass, one 65-row aug pass [kh;ones] x [qh;-rowmax]). The ternary weights
are exactly representable in fp16. The row max comes from a separate fp16
stats pass in [row,key] layout (masked diag block, DVE row-reduce),
negated+transposed back via a padded matmul against -I (columns at
stride 32 so the rows land on legal partition offsets). exp on ACT to
fp16 P; causal zeroing of the diagonal P block runs on GPSIMD post-exp;
V is single fp16 in [token,feat] layout with a ones column that yields
softmax denominators inside the PV matmul; normalization multiplies by a
GPSIMD partition-broadcast of the reciprocal denominator row.

Scheduling: the row-max stats for row-chunk 0 are software-pipelined into
the projection phase; in the attention phase a flat (row-chunk, head)
pipeline overlaps each step's PE work with the stats reduces of the step
three ahead (emission order: augrow(i+1), attn(i), stats(i+3)), keeping
DVE (reduces) and PE (matmuls) concurrently busy.
"""

import numpy as np
import ml_dtypes

import concourse.bass as bass
import concourse.bacc as bacc
import concourse.mybir as mybir
import concourse.tile as tile
from concourse.bass_utils import run_bass_kernel_spmd
from concourse.masks import make_identity

f32 = mybir.dt.float32
f16 = mybir.dt.float16

EPS = np.float32(1.1920929e-07)
B, T, D = 2, 2048, 1024
H, HKV, HD = 16, 4, 64
NH = 4            # local (per-core) query heads
LF = NH * HD      # 256 local q features
P = 128
DT = D // P       # 8 d-tiles
CH = 4            # 512-wide token chunks
CW = 512
QB = T // P       # 16 query row blocks
KB = T // P       # 16 key blocks
NEG = -1.0e30

Exp = mybir.ActivationFunctionType.Exp
AOp = mybir.AluOpType
AxX = mybir.AxisListType.X

# 1 = x in single fp16 (q/k pairs absorb projection rounding only);
# 2 = x as fp16 hi/lo pair (22-bit x, bulletproof precision)
X_PASSES = 2


def _build():
    nc = bacc.Bacc("TRN2", target_bir_lowering=False, debug=False)

    xh_d = nc.dram_tensor("xh", [D, T], f16, kind="ExternalInput").ap()
    if X_PASSES == 2:
        xl_d = nc.dram_tensor("xl", [D, T], f16, kind="ExternalInput").ap()
    std_d = nc.dram_tensor("stdc", [P, KB], f32, kind="ExternalInput").ap()
    wq = nc.dram_tensor("wq", [D, LF], f16, kind="ExternalInput").ap()
    wkv = nc.dram_tensor("wkv", [D, P], f16, kind="ExternalInput").ap()
    wo = nc.dram_tensor("wo", [LF, D], f16, kind="ExternalInput").ap()
    yp = nc.dram_tensor("yp", [T, D], f32, kind="ExternalOutput").ap()
    ssqa = nc.dram_tensor("ssqa", [1, T], f32, kind="ExternalOutput").ap()

    with tile.TileContext(nc) as tc:
        with (
            tc.tile_pool(name="const", bufs=1) as const,
            tc.tile_pool(name="persist", bufs=1) as persist,
        ):
            ident16 = const.tile([P, P], f16, tag="ident16")
            make_identity(nc, ident16[:])
            negident = const.tile([P, P], f16, tag="negident")
            make_identity(nc, negident[:])
            nc.gpsimd.tensor_scalar_mul(negident[:], negident[:], -1.0)
            # stats mask, positioned in the LAST 128 cols of a 512 window
            # (S layout [row, key]): key > row -> NEG
            mask512 = const.tile([P, CW], f32, tag="mask512")
            nc.gpsimd.memset(mask512[:], 0.0)
            nc.gpsimd.affine_select(
                out=mask512[:, CW - P:CW], in_=mask512[:, CW - P:CW],
                compare_op=AOp.is_ge, fill=NEG,
                base=0, pattern=[[-1, P]], channel_multiplier=1,
            )

            stdc = persist.tile([P, KB], f32, tag="stdc")
            nc.sync.dma_start(out=stdc[:], in_=std_d[:])
            wq_sb = persist.tile([P, DT, LF], f16, tag="wq_sb")
            nc.sync.dma_start(
                out=wq_sb[:], in_=wq.rearrange("(dt p) f -> p dt f", p=P)
            )
            wkv_sb = persist.tile([P, DT, P], f16, tag="wkv_sb")
            nc.sync.dma_start(
                out=wkv_sb[:], in_=wkv.rearrange("(dt p) f -> p dt f", p=P)
            )
            wo_sb = persist.tile([P, 2, D], f16, tag="wo_sb")
            nc.sync.dma_start(
                out=wo_sb[:], in_=wo.rearrange("(ft p) o -> p ft o", p=P)
            )

            # QTaug = [qh; -max], QTs = [ql; qh]; KTaug = [kh; ones],
            # KTs = [kh; kl].  The lo parts absorb the psum->fp16 rounding
            # (and, with X_PASSES == 2, the x rounding as well).
            QTaug = [persist.tile([65, T], f16, tag=f"qta{h}", name=f"qta{h}")
                     for h in range(NH)]
            QTs = [persist.tile([P, T], f16, tag=f"qts{h}", name=f"qts{h}")
                   for h in range(NH)]
            KTaug = persist.tile([65, T], f16, tag="kta")
            KTs = persist.tile([P, T], f16, tag="kts")
            nc.gpsimd.memset(KTaug[64:65, :], 1.0)
            # V in transposed [token, feat] layout, fp16, col 64 = ones
            Vsb = [persist.tile([P, 65], f16, tag=f"v{kb}", name=f"v{kb}")
                   for kb in range(KB)]
            for kb in range(KB):
                nc.gpsimd.memset(Vsb[kb][:, 64:65], 1.0)
            aTh = [persist.tile([P, T], f16, tag=f"ath{ft}", name=f"ath{ft}")
                   for ft in range(2)]
            onesc = persist.tile([P, 1], f16, tag="onesc")
            nc.gpsimd.memset(onesc[:], 1.0)
            ssqrow = persist.tile([1, T], f32, tag="ssqrow")

            # --- stats helpers: software-pipelined masked row maxes ---
            # mallA holds per-head max columns at stride 32 so the
            # transposed rows land on legal partition offsets (0/32/64/96)
            def emit_stats(mstat, ps_stat, rc, h, mallA_h):
                nc.gpsimd.memset(mallA_h[:], 0.0)
                for j in range(4):
                    qb = rc * 4 + j
                    nk = (qb + 1) * P
                    nchunks = (nk + CW - 1) // CW
                    mxs = mstat.tile([P, 4], f32, tag="mxs")
                    for ic in range(nchunks):
                        k0 = ic * CW
                        kw = min(CW, nk - k0)
                        last = ic == nchunks - 1
                        pS = ps_stat.tile([P, CW], f32, tag="pstat")
                        nc.tensor.matmul(
                            pS[:, :kw],
                            QTaug[h][0:HD, qb * P:(qb + 1) * P],
                            KTaug[0:HD, k0:k0 + kw],
                            start=True, stop=True,
                        )
                        if last:
                            # diag-block mask add, then row max
                            nc.vector.tensor_add(
                                pS[:, kw - P:kw], pS[:, kw - P:kw],
                                mask512[:, CW - P:CW])
                            dst = (mallA_h[:, 32 * j:32 * j + 1]
                                   if nchunks == 1 else mxs[:, ic:ic + 1])
                            nc.vector.reduce_max(dst, pS[:, :kw], axis=AxX)
                        else:
                            nc.vector.reduce_max(mxs[:, ic:ic + 1],
                                                 pS[:, :kw], axis=AxX)
                    if nchunks > 1:
                        nc.vector.reduce_max(mallA_h[:, 32 * j:32 * j + 1],
                                             mxs[:, 0:nchunks], axis=AxX)

            def emit_augrow(ps_nm, rc, h, mallA_h):
                # negated transpose of the row maxes for one head
                pnm = ps_nm.tile([97, P], f32, tag="pnm",
                                 name=f"pnm_{rc}_{h}")
                nc.tensor.matmul(pnm[:], mallA_h[:], negident[:],
                                 start=True, stop=True)
                for j in range(4):
                    qb = rc * 4 + j
                    row = pnm[32 * j:32 * j + 1, :]
                    dst = QTaug[h][HD:HD + 1, qb * P:(qb + 1) * P]
                    nc.scalar.copy(dst, row)

            with tc.tile_pool(name="mstat", bufs=2) as mstat:
                mallA = {}

                def alloc_mallA(rc, h):
                    mallA[(rc, h)] = mstat.tile(
                        [P, 97], f16, tag=f"malla{h}",
                        name=f"malla_{rc}_{h}", bufs=3)
                    return mallA[(rc, h)]

                # -------------- phase 2: projections + stats(0) --------------
                with (
                    tc.tile_pool(name="ph2", bufs=4) as ph2,
                    tc.tile_pool(name="ph2v", bufs=2) as ph2v,
                    tc.tile_pool(name="ps_q", bufs=3, space="PSUM") as ps_q,
                    tc.tile_pool(name="ps_kv", bufs=2, space="PSUM") as ps_kv,
                    tc.tile_pool(name="ps_vt", bufs=1, space="PSUM") as ps_vt,
                    tc.tile_pool(name="ps_st0", bufs=2, space="PSUM") as ps_st0,
                ):
                    for c in range(CH):
                        cs = slice(c * CW, (c + 1) * CW)
                        psQ = [ps_q.tile([P, CW], f32, tag="psq",
                                         name=f"psq_{c}_{i}") for i in range(2)]
                        psKV = ps_kv.tile([P, CW], f32, tag="pskv")
                        for d in range(DT):
                            th = ph2.tile([P, CW], f16, tag="th")
                            nc.sync.dma_start(out=th[:],
                                              in_=xh_d[d * P:(d + 1) * P, cs])
                            xparts = [th]
                            if X_PASSES == 2:
                                tl = ph2.tile([P, CW], f16, tag="tl")
                                nc.sync.dma_start(
                                    out=tl[:], in_=xl_d[d * P:(d + 1) * P, cs])
                                xparts.append(tl)
                            first, last = d == 0, d == DT - 1
                            for ft in range(2):
                                for xi, tx in enumerate(xparts):
                                    nc.tensor.matmul(
                                        psQ[ft][:],
                                        wq_sb[:, d, ft * P:(ft + 1) * P], tx[:],
                                        start=first and xi == 0,
                                        stop=last and xi == len(xparts) - 1)
                            for xi, tx in enumerate(xparts):
                                nc.tensor.matmul(
                                    psKV[:], wkv_sb[:, d, :], tx[:],
                                    start=first and xi == 0,
                                    stop=last and xi == len(xparts) - 1)
                        for ft in range(2):
                            for sub in range(2):
                                h = 2 * ft + sub
                                pslc = psQ[ft][sub * HD:(sub + 1) * HD, :]
                                nc.scalar.copy(QTaug[h][0:HD, cs], pslc)
                                nc.gpsimd.tensor_copy(QTs[h][HD:P, cs],
                                                      QTaug[h][0:HD, cs])
                                nc.vector.tensor_sub(QTs[h][0:HD, cs], pslc,
                                                     QTaug[h][0:HD, cs])
                        nc.scalar.copy(KTaug[0:HD, cs], psKV[0:HD, :])
                        nc.gpsimd.tensor_copy(KTs[0:HD, cs], KTaug[0:HD, cs])
                        nc.vector.tensor_sub(KTs[HD:P, cs], psKV[0:HD, :],
                                             KTaug[0:HD, cs])
                        # V (rows 64:128 of psKV) -> fp16, transpose to
                        # [token, feat], un-normalize by per-token std
                        vt = ph2v.tile([HD, CW], f16, tag="vt")
                        nc.vector.tensor_copy(vt[:], psKV[HD:P, :])
                        for s4 in range(4):
                            kb = c * 4 + s4
                            pvt = ps_vt.tile([P, HD], f16, tag="pvt")
                            nc.tensor.transpose(pvt[:],
                                                vt[:, s4 * P:(s4 + 1) * P],
                                                ident16[0:HD, 0:HD])
                            nc.scalar.mul(Vsb[kb][:, 0:HD], pvt[:],
                                          stdc[:, kb:kb + 1])
                        # stats(rc=0, head c) rides along with chunk c
                        emit_stats(mstat, ps_st0, 0, c, alloc_mallA(0, c))

                # ------ phases 3-6: attention + O-proj, per row-chunk ------
                with (
                    tc.tile_pool(name="ph3", bufs=6) as ph3,
                    tc.tile_pool(name="ptp", bufs=1) as ptp,
                    tc.tile_pool(name="ph6", bufs=4) as ph6,
                    tc.tile_pool(name="ps_stat", bufs=2, space="PSUM") as ps_stat,
                    tc.tile_pool(name="ps_st", bufs=2, space="PSUM") as ps_st,
                    tc.tile_pool(name="ps_nm", bufs=1, space="PSUM") as ps_nm,
                    tc.tile_pool(name="ps_y", bufs=2, space="PSUM") as ps_y,
                ):
                    PTt = [ptp.tile([P, CW], f16, tag=f"pt{kc}", name=f"pt{kc}")
                           for kc in range(KB)]
                    # flat (rc, h) pipeline with one-step stats lookahead:
                    # attn(step i) overlaps stats+augrow of step i+1, whose
                    # DVE size tracks step i's PE size
                    steps = [(rc, h) for rc in range(4) for h in range(NH)]
                    emit_augrow(ps_nm, 0, 0, mallA[(0, 0)])
                    for i, (rc, h) in enumerate(steps):
                        rs = slice(rc * CW, (rc + 1) * CW)
                        # augrow for the NEXT step first so its small copies
                        # sit early in the DVE/ACT queues
                        if i + 1 < len(steps):
                            rcn, hn = steps[i + 1]
                            emit_augrow(ps_nm, rcn, hn, mallA[(rcn, hn)])
                        attention_head(nc, tc, ph3, ptp, ph6, ps_st, ps_y,
                                       PTt, QTaug, QTs, KTaug, KTs, Vsb,
                                       aTh, rc, h, rs)
                        # three-step stats lookahead
                        if i == 0:
                            for ii in (2, 3):
                                rc2, h2 = steps[ii]
                                if rc2 >= 1:
                                    emit_stats(mstat, ps_stat, rc2, h2,
                                               alloc_mallA(rc2, h2))
                        elif i + 3 < len(steps):
                            rc2, h2 = steps[i + 3]
                            if rc2 >= 1:
                                emit_stats(mstat, ps_stat, rc2, h2,
                                           alloc_mallA(rc2, h2))
                        if h == NH - 1:
                            # ssq of the normalized attn rows + O-projection
                            psq = ps_y.tile([1, CW], f32, tag="psy",
                                            name=f"psq_{rc}")
                            for ft in range(2):
                                sqt = ph6.tile([P, CW], f16, tag="sqt")
                                nc.gpsimd.tensor_mul(sqt[:], aTh[ft][:, rs],
                                                     aTh[ft][:, rs])
                                nc.tensor.matmul(psq[:], onesc[:], sqt[:],
                                                 start=(ft == 0),
                                                 stop=(ft == 1))
                            nc.vector.tensor_copy(ssqrow[0:1, rs], psq[:])
                            for j in range(4):
                                qb = rc * 4 + j
                                qs = slice(qb * P, (qb + 1) * P)
                                for oc in range(2):
                                    os_ = slice(oc * CW, (oc + 1) * CW)
                                    psY = ps_y.tile([P, CW], f32, tag="psy",
                                                    name=f"psY_{qb}_{oc}")
                                    for ft in range(2):
                                        nc.tensor.matmul(
                                            psY[:], aTh[ft][:, qs],
                                            wo_sb[:, ft, os_],
                                            start=(ft == 0), stop=(ft == 1))
                                    ysb = ph6.tile([P, CW], f32, tag="ysb")
                                    if rc == 3:
                                        nc.vector.tensor_copy(ysb[:], psY[:])
                                    else:
                                        nc.scalar.copy(ysb[:], psY[:])
                                    nc.sync.dma_start(out=yp[qs, os_],
                                                      in_=ysb[:])
                    nc.sync.dma_start(out=ssqa[:], in_=ssqrow[:])

    nc.finalize()
    return nc


def attention_head(nc, tc, ph3, ptp, ph6, ps_st, ps_y, PTt, QTaug, QTs,
                   KTaug, KTs, Vsb, aTh, rc, h, rs):
    # S.T for this row chunk: 2-pass (cross terms via [kh;kl]x[ql;qh],
    # hi terms + -max via the 65-row aug pass) + exp; diagonal P block
    # zeroed on GPSIMD post-exp
    for kc in range((rc + 1) * 4):
        c0 = max(kc * P, rc * CW)
        cw_ = (rc + 1) * CW - c0
        kslc = slice(kc * P, (kc + 1) * P)
        cslc = slice(c0, c0 + cw_)
        lo = c0 - rc * CW
        pST = ps_st.tile([P, CW], f32, tag="pst")
        nc.tensor.matmul(pST[:, lo:lo + cw_], KTs[:, kslc],
                         QTs[h][:, cslc], start=True, stop=False)
        nc.tensor.matmul(pST[:, lo:lo + cw_], KTaug[:, kslc],
                         QTaug[h][:, cslc], start=False, stop=True)
        nc.scalar.activation(PTt[kc][:, lo:lo + cw_],
                             pST[:, lo:lo + cw_], Exp)
        if c0 == kc * P:
            # zero P above the diagonal: keep row >= key
            nc.gpsimd.affine_select(
                out=PTt[kc][:, lo:lo + P], in_=PTt[kc][:, lo:lo + P],
                compare_op=AOp.is_ge, fill=0.0,
                base=0, pattern=[[1, P]], channel_multiplier=-1,
            )

    # transposed PV: psO rows = [PV | denom]
    psO = ps_st.tile([65, CW], f32, tag="pso", bufs=1)
    for kc in range((rc + 1) * 4):
        c0 = max(kc * P, rc * CW)
        cw_ = (rc + 1) * CW - c0
        lo = c0 - rc * CW
        nc.tensor.matmul(
            psO[:, lo:lo + cw_], Vsb[kc][:, 0:65], PTt[kc][:, lo:lo + cw_],
            start=(kc == 0), stop=(kc == (rc + 1) * 4 - 1),
        )
    # normalize columns by 1/denom via gpsimd broadcast
    rd32 = ph3.tile([1, CW], f32, tag="rd32")
    nc.vector.reciprocal(rd32[:], psO[64:65, :])
    dbcS = ph3.tile([HD, CW], f32, tag="dbcs")
    nc.gpsimd.partition_broadcast(dbcS[:], rd32[:], channels=HD)
    ft, sub = h // 2, h % 2
    nc.vector.tensor_mul(aTh[ft][sub * HD:(sub + 1) * HD, rs],
                         psO[0:HD, :], dbcS[:])


def _ternary(w):
    th = np.abs(w).mean(dtype=np.float64)
    return (np.sign(w) * (np.abs(w) > th)).astype(np.float32)


_CACHE = {}


def kernel(x, q_w, q_g, k_w, k_g, v_w, o_w, o_g, qk_gain):
    x = np.asarray(x, np.float32)
    wq_eff = (_ternary(np.asarray(q_w)) * np.asarray(q_g)[None, :]
              * np.float32(qk_gain) / np.float32(np.sqrt(np.float32(HD))))
    wk_eff = _ternary(np.asarray(k_w)) * np.asarray(k_g)[None, :]
    wo_eff = _ternary(np.asarray(o_w)) * np.asarray(o_g)[None, :]
    wqT = np.ascontiguousarray(wq_eff.T).astype(np.float16)      # [D, H*HD]
    wkT = wk_eff.T.astype(np.float16)                            # [D, HKV*HD]
    wvT = np.asarray(v_w, np.float32).T.astype(np.float16)
    woT = np.ascontiguousarray(wo_eff.T).astype(np.float16)      # [D, D]

    # per-token rms scales (host); x_hat = x * r, V un-normalized on device
    # by std = 1/r
    xs = x.astype(np.float64)
    ssq = (xs * xs).mean(-1) + np.float64(EPS)
    r = (1.0 / np.sqrt(ssq)).astype(np.float32)                  # [B, T]
    std = np.sqrt(ssq).astype(np.float32)                        # [B, T]
    xn = (x * r[:, :, None]).astype(np.float32)

    if "nc" not in _CACHE:
        _CACHE["nc"] = _build()
    nc = _CACHE["nc"]

    in_maps = []
    for core in range(8):
        b, g = divmod(core, 4)
        xnT = np.ascontiguousarray(xn[b].T)                      # [D, T] f32
        xh = xnT.astype(np.float16)
        wkv_c = np.concatenate(
            [wkT[:, g * HD:(g + 1) * HD], wvT[:, g * HD:(g + 1) * HD]], axis=1)
        im = {
            "xh": xh,
            "stdc": np.ascontiguousarray(std[b].reshape(KB, P).T),
            "wq": np.ascontiguousarray(wqT[:, g * LF:(g + 1) * LF]),
            "wkv": np.ascontiguousarray(wkv_c),
            "wo": np.ascontiguousarray(woT[g * LF:(g + 1) * LF, :]),
        }
        if X_PASSES == 2:
            im["xl"] = (xnT - xh.astype(np.float32)).astype(np.float16)
        in_maps.append(im)
    _CACHE["in_maps"] = in_maps
    res = run_bass_kernel_spmd(nc, in_maps, list(range(8)))

    out = np.empty((B, T, D), np.float32)
    for b in range(B):
        ssq_a = np.zeros((T,), np.float32)
        ysum = np.zeros((T, D), np.float32)
        for g in range(4):
            rr = res.results[b * 4 + g]
            ysum += rr["yp"]
            ssq_a += rr["ssqa"][0]
        ro = 1.0 / np.sqrt(ssq_a / np.float32(D) + EPS)
        out[b] = ysum * ro[:, None]
    return out


if __name__ == "__main__":
    data = np.load("/root/problem/inputs.npz")
    out = kernel(**{k: data[k] for k in data.files})
    ref = np.load("/root/problem/ref_out.npy")
    d = out.astype(np.float64) - ref.astype(np.float64)
    rv = (d * d).sum() / (ref.astype(np.float64) ** 2).sum()
    print("resid_var=%.3e relerr=%.3e absmax=%.3g" %
          (rv, np.sqrt(rv), np.abs(d).max()))


# revision 42
# speedup vs baseline: 1.0856x; 1.0050x over previous
"""TRN2 Bass kernel for nn_AttentionCell (BitLinear GQA attention cell).

Sharding (8 cores): data-parallel over batch (2) x tensor-parallel over the
4 KV head-groups (4 query heads each). Each core computes Q/K/V projections,
causal softmax attention for its 4 heads, and a row-parallel partial of the
output projection; the host sums the 4 partials per batch and applies the
final RMSNorm row scale.

Precision: x streams as an fp16 hi/lo pair (~22-bit); Q and K are fp16
pairs ([ql;qh]/[kh;kl]); scores use a 2-pass matmul (one 128-row cross
pass, one 65-row aug pass [kh;ones] x [qh;-rowmax]). The ternary weights
are exactly representable in fp16. The row max comes from a separate fp16
stats pass in [row,key] layout (masked diag block, DVE row-reduce),
negated+transposed back via a padded matmul against -I (columns at
stride 32 so the rows land on legal partition offsets). exp on ACT to
fp16 P; causal zeroing of the diagonal P block runs on GPSIMD post-exp;
V is single fp16 in [token,feat] layout with a ones column that yields
softmax denominators inside the PV matmul; normalization multiplies by a
GPSIMD partition-broadcast of the reciprocal denominator row.

Scheduling: the row-max stats for row-chunk 0 are software-pipelined into
the projection phase; in the attention phase a flat (row-chunk, head)
pipeline overlaps each step's PE work with the stats reduces of the step
three ahead (emission order: augrow(i+1), attn(i), stats(i+3)), keeping
DVE (reduces) and PE (matmuls) concurrently busy.
"""

import numpy as np
import ml_dtypes

import concourse.bass as bass
import concourse.bacc as bacc
import concourse.mybir as mybir
import concourse.tile as tile
from concourse.bass_utils import run_bass_kernel_spmd
from concourse.masks import make_identity

f32 = mybir.dt.float32
f16 = mybir.dt.float16

EPS = np.float32(1.1920929e-07)
B, T, D = 2, 2048, 1024
H, HKV, HD = 16, 4, 64
NH = 4            # local (per-core) query heads
LF = NH * HD      # 256 local q features
P = 128
DT = D // P       # 8 d-tiles
CH = 4            # 512-wide token chunks
CW = 512
QB = T // P       # 16 query row blocks
KB = T // P       # 16 key blocks
NEG = -1.0e30

Exp = mybir.ActivationFunctionType.Exp
AOp = mybir.AluOpType
AxX = mybir.AxisListType.X

# 1 = x in single fp16 (q/k pairs absorb projection rounding only);
# 2 = x as fp16 hi/lo pair (22-bit x, bulletproof precision)
X_PASSES = 2


def _build():
    nc = bacc.Bacc("TRN2", target_bir_lowering=False, debug=False)

    xh_d = nc.dram_tensor("xh", [D, T], f16, kind="ExternalInput").ap()
    if X_PASSES == 2:
        xl_d = nc.dram_tensor("xl", [D, T], f16, kind="ExternalInput").ap()
    std_d = nc.dram_tensor("stdc", [P, KB], f32, kind="ExternalInput").ap()
    wq = nc.dram_tensor("wq", [D, LF], f16, kind="ExternalInput").ap()
    wkv = nc.dram_tensor("wkv", [D, P], f16, kind="ExternalInput").ap()
    wo = nc.dram_tensor("wo", [LF, D], f16, kind="ExternalInput").ap()
    yp = nc.dram_tensor("yp", [T, D], f32, kind="ExternalOutput").ap()
    ssqa = nc.dram_tensor("ssqa", [1, T], f32, kind="ExternalOutput").ap()

    with tile.TileContext(nc) as tc:
        with (
            tc.tile_pool(name="const", bufs=1) as const,
            tc.tile_pool(name="persist", bufs=1) as persist,
        ):
            ident16 = const.tile([P, P], f16, tag="ident16")
            make_identity(nc, ident16[:])
            negident = const.tile([P, P], f16, tag="negident")
            make_identity(nc, negident[:])
            nc.gpsimd.tensor_scalar_mul(negident[:], negident[:], -1.0)
            # stats mask, positioned in the LAST 128 cols of a 512 window
            # (S layout [row, key]): key > row -> NEG
            mask512 = const.tile([P, CW], f32, tag="mask512")
            nc.gpsimd.memset(mask512[:], 0.0)
            nc.gpsimd.affine_select(
                out=mask512[:, CW - P:CW], in_=mask512[:, CW - P:CW],
                compare_op=AOp.is_ge, fill=NEG,
                base=0, pattern=[[-1, P]], channel_multiplier=1,
            )

            stdc = persist.tile([P, KB], f32, tag="stdc")
            nc.sync.dma_start(out=stdc[:], in_=std_d[:])
            wq_sb = persist.tile([P, DT, LF], f16, tag="wq_sb")
            nc.sync.dma_start(
                out=wq_sb[:], in_=wq.rearrange("(dt p) f -> p dt f", p=P)
            )
            wkv_sb = persist.tile([P, DT, P], f16, tag="wkv_sb")
            nc.sync.dma_start(
                out=wkv_sb[:], in_=wkv.rearrange("(dt p) f -> p dt f", p=P)
            )
            wo_sb = persist.tile([P, 2, D], f16, tag="wo_sb")
            nc.sync.dma_start(
                out=wo_sb[:], in_=wo.rearrange("(ft p) o -> p ft o", p=P)
            )

            # QTaug = [qh; -max], QTs = [ql; qh]; KTaug = [kh; ones],
            # KTs = [kh; kl].  The lo parts absorb the psum->fp16 rounding
            # (and, with X_PASSES == 2, the x rounding as well).
            QTaug = [persist.tile([65, T], f16, tag=f"qta{h}", name=f"qta{h}")
                     for h in range(NH)]
            QTs = [persist.tile([P, T], f16, tag=f"qts{h}", name=f"qts{h}")
                   for h in range(NH)]
            KTaug = persist.tile([65, T], f16, tag="kta")
            KTs = persist.tile([P, T], f16, tag="kts")
            nc.gpsimd.memset(KTaug[64:65, :], 1.0)
            # V in transposed [token, feat] layout, fp16, col 64 = ones
            Vsb = [persist.tile([P, 65], f16, tag=f"v{kb}", name=f"v{kb}")
                   for kb in range(KB)]
            for kb in range(KB):
                nc.gpsimd.memset(Vsb[kb][:, 64:65], 1.0)
            aTh = [persist.tile([P, T], f16, tag=f"ath{ft}", name=f"ath{ft}")
                   for ft in range(2)]
            onesc = persist.tile([P, 1], f16, tag="onesc")
            nc.gpsimd.memset(onesc[:], 1.0)
            ssqrow = persist.tile([1, T], f32, tag="ssqrow")

            # --- stats helpers: software-pipelined masked row maxes ---
            # mallA holds per-head max columns at stride 32 so the
            # transposed rows land on legal partition offsets (0/32/64/96)
            def emit_stats(mstat, ps_stat, rc, h, mallA_h):
                nc.gpsimd.memset(mallA_h[:], 0.0)
                for j in range(4):
                    qb = rc * 4 + j
                    nk = (qb + 1) * P
                    nchunks = (nk + CW - 1) // CW
                    mxs = mstat.tile([P, 4], f32, tag="mxs")
                    for ic in range(nchunks):
                        k0 = ic * CW
                        kw = min(CW, nk - k0)
                        last = ic == nchunks - 1
                        pS = ps_stat.tile([P, CW], f32, tag="pstat")
                        nc.tensor.matmul(
                            pS[:, :kw],
                            QTaug[h][0:HD, qb * P:(qb + 1) * P],
                            KTaug[0:HD, k0:k0 + kw],
                            start=True, stop=True,
                        )
                        if last:
                            # diag-block mask add, then row max
                            nc.vector.tensor_add(
                                pS[:, kw - P:kw], pS[:, kw - P:kw],
                                mask512[:, CW - P:CW])
                            dst = (mallA_h[:, 32 * j:32 * j + 1]
                                   if nchunks == 1 else mxs[:, ic:ic + 1])
                            nc.vector.reduce_max(dst, pS[:, :kw], axis=AxX)
                        else:
                            nc.vector.reduce_max(mxs[:, ic:ic + 1],
                                                 pS[:, :kw], axis=AxX)
                    if nchunks > 1:
                        nc.vector.reduce_max(mallA_h[:, 32 * j:32 * j + 1],
                                             mxs[:, 0:nchunks], axis=AxX)

            def emit_augrow(ps_nm, rc, h, mallA_h):
                # negated transpose of the row maxes for one head
                pnm = ps_nm.tile([97, P], f32, tag="pnm",
                                 name=f"pnm_{rc}_{h}")
                nc.tensor.matmul(pnm[:], mallA_h[:], negident[:],
                                 start=True, stop=True)
                for j in range(4):
                    qb = rc * 4 + j
                    row = pnm[32 * j:32 * j + 1, :]
                    dst = QTaug[h][HD:HD + 1, qb * P:(qb + 1) * P]
                    nc.scalar.copy(dst, row)

            with tc.tile_pool(name="mstat", bufs=2) as mstat:
                mallA = {}

                def alloc_mallA(rc, h):
                    mallA[(rc, h)] = mstat.tile(
                        [P, 97], f16, tag=f"malla{h}",
                        name=f"malla_{rc}_{h}", bufs=3)
                    return mallA[(rc, h)]

                # -------------- phase 2: projections + stats(0) --------------
                with (
                    tc.tile_pool(name="ph2", bufs=8) as ph2,
                    tc.tile_pool(name="ph2v", bufs=2) as ph2v,
                    tc.tile_pool(name="ps_q", bufs=3, space="PSUM") as ps_q,
                    tc.tile_pool(name="ps_kv", bufs=2, space="PSUM") as ps_kv,
                    tc.tile_pool(name="ps_vt", bufs=1, space="PSUM") as ps_vt,
                    tc.tile_pool(name="ps_st0", bufs=2, space="PSUM") as ps_st0,
                ):
                    for c in range(CH):
                        cs = slice(c * CW, (c + 1) * CW)
                        psQ = [ps_q.tile([P, CW], f32, tag="psq",
                                         name=f"psq_{c}_{i}") for i in range(2)]
                        psKV = ps_kv.tile([P, CW], f32, tag="pskv")
                        for d in range(DT):
                            th = ph2.tile([P, CW], f16, tag="th")
                            nc.sync.dma_start(out=th[:],
                                              in_=xh_d[d * P:(d + 1) * P, cs])
                            xparts = [th]
                            if X_PASSES == 2:
                                tl = ph2.tile([P, CW], f16, tag="tl")
                                nc.sync.dma_start(
                                    out=tl[:], in_=xl_d[d * P:(d + 1) * P, cs])
                                xparts.append(tl)
                            first, last = d == 0, d == DT - 1
                            for ft in range(2):
                                for xi, tx in enumerate(xparts):
                                    nc.tensor.matmul(
                                        psQ[ft][:],
                                        wq_sb[:, d, ft * P:(ft + 1) * P], tx[:],
                                        start=first and xi == 0,
                                        stop=last and xi == len(xparts) - 1)
                            for xi, tx in enumerate(xparts):
                                nc.tensor.matmul(
                                    psKV[:], wkv_sb[:, d, :], tx[:],
                                    start=first and xi == 0,
                                    stop=last and xi == len(xparts) - 1)
                        for ft in range(2):
                            for sub in range(2):
                                h = 2 * ft + sub
                                pslc = psQ[ft][sub * HD:(sub + 1) * HD, :]
                                nc.scalar.copy(QTaug[h][0:HD, cs], pslc)
                                nc.gpsimd.tensor_copy(QTs[h][HD:P, cs],
                                                      QTaug[h][0:HD, cs])
                                nc.vector.tensor_sub(QTs[h][0:HD, cs], pslc,
                                                     QTaug[h][0:HD, cs])
                        nc.scalar.copy(KTaug[0:HD, cs], psKV[0:HD, :])
                        nc.gpsimd.tensor_copy(KTs[0:HD, cs], KTaug[0:HD, cs])
                        nc.vector.tensor_sub(KTs[HD:P, cs], psKV[0:HD, :],
                                             KTaug[0:HD, cs])
                        # V (rows 64:128 of psKV) -> fp16, transpose to
                        # [token, feat], un-normalize by per-token std
                        vt = ph2v.tile([HD, CW], f16, tag="vt")
                        nc.scalar.copy(vt[:], psKV[HD:P, :])
                        for s4 in range(4):
                            kb = c * 4 + s4
                            pvt = ps_vt.tile([P, HD], f16, tag="pvt")
                            nc.tensor.transpose(pvt[:],
                                                vt[:, s4 * P:(s4 + 1) * P],
                                                ident16[0:HD, 0:HD])
                            nc.scalar.mul(Vsb[kb][:, 0:HD], pvt[:],
                                          stdc[:, kb:kb + 1])
                        # stats(rc=0, head c) rides along with chunk c
                        emit_stats(mstat, ps_st0, 0, c, alloc_mallA(0, c))

                # ------ phases 3-6: attention + O-proj, per row-chunk ------
                with (
                    tc.tile_pool(name="ph3", bufs=6) as ph3,
                    tc.tile_pool(name="ptp", bufs=1) as ptp,
                    tc.tile_pool(name="ph6", bufs=4) as ph6,
                    tc.tile_pool(name="ps_stat", bufs=2, space="PSUM") as ps_stat,
                    tc.tile_pool(name="ps_st", bufs=2, space="PSUM") as ps_st,
                    tc.tile_pool(name="ps_nm", bufs=1, space="PSUM") as ps_nm,
                    tc.tile_pool(name="ps_y", bufs=2, space="PSUM") as ps_y,
                ):
                    PTt = [ptp.tile([P, CW], f16, tag=f"pt{kc}", name=f"pt{kc}")
                           for kc in range(KB)]
                    # flat (rc, h) pipeline with one-step stats lookahead:
                    # attn(step i) overlaps stats+augrow of step i+1, whose
                    # DVE size tracks step i's PE size
                    steps = [(rc, h) for rc in range(4) for h in range(NH)]
                    emit_augrow(ps_nm, 0, 0, mallA[(0, 0)])
                    for i, (rc, h) in enumerate(steps):
                        rs = slice(rc * CW, (rc + 1) * CW)
                        # augrow for the NEXT step first so its small copies
                        # sit early in the DVE/ACT queues
                        if i + 1 < len(steps):
                            rcn, hn = steps[i + 1]
                            emit_augrow(ps_nm, rcn, hn, mallA[(rcn, hn)])
                        attention_head(nc, tc, ph3, ptp, ph6, ps_st, ps_y,
                                       PTt, QTaug, QTs, KTaug, KTs, Vsb,
                                       aTh, rc, h, rs)
                        # three-step stats lookahead
                        if i == 0:
                            for ii in (2, 3):
                                rc2, h2 = steps[ii]
                                if rc2 >= 1:
                                    emit_stats(mstat, ps_stat, rc2, h2,
                                               alloc_mallA(rc2, h2))
                        elif i + 3 < len(steps):
                            rc2, h2 = steps[i + 3]
                            if rc2 >= 1:
                                emit_stats(mstat, ps_stat, rc2, h2,
                                           alloc_mallA(rc2, h2))
                        if h == NH - 1:
                            # ssq of the normalized attn rows + O-projection
                            psq = ps_y.tile([1, CW], f32, tag="psy",
                                            name=f"psq_{rc}")
                            for ft in range(2):
                                sqt = ph6.tile([P, CW], f16, tag="sqt")
                                nc.gpsimd.tensor_mul(sqt[:], aTh[ft][:, rs],
                                                     aTh[ft][:, rs])
                                nc.tensor.matmul(psq[:], onesc[:], sqt[:],
                                                 start=(ft == 0),
                                                 stop=(ft == 1))
                            nc.vector.tensor_copy(ssqrow[0:1, rs], psq[:])
                            for j in range(4):
                                qb = rc * 4 + j
                                qs = slice(qb * P, (qb + 1) * P)
                                for oc in range(2):
                                    os_ = slice(oc * CW, (oc + 1) * CW)
                                    psY = ps_y.tile([P, CW], f32, tag="psy",
                                                    name=f"psY_{qb}_{oc}")
                                    for ft in range(2):
                                        nc.tensor.matmul(
                                            psY[:], aTh[ft][:, qs],
                                            wo_sb[:, ft, os_],
                                            start=(ft == 0), stop=(ft == 1))
                                    ysb = ph6.tile([P, CW], f32, tag="ysb")
                                    if rc == 3:
                                        nc.vector.tensor_copy(ysb[:], psY[:])
                                    else:
                                        nc.scalar.copy(ysb[:], psY[:])
                                    nc.sync.dma_start(out=yp[qs, os_],
                                                      in_=ysb[:])
                    nc.sync.dma_start(out=ssqa[:], in_=ssqrow[:])

    nc.finalize()
    return nc


def attention_head(nc, tc, ph3, ptp, ph6, ps_st, ps_y, PTt, QTaug, QTs,
                   KTaug, KTs, Vsb, aTh, rc, h, rs):
    # S.T for this row chunk: 2-pass (cross terms via [kh;kl]x[ql;qh],
    # hi terms + -max via the 65-row aug pass) + exp; diagonal P block
    # zeroed on GPSIMD post-exp
    for kc in range((rc + 1) * 4):
        c0 = max(kc * P, rc * CW)
        cw_ = (rc + 1) * CW - c0
        kslc = slice(kc * P, (kc + 1) * P)
        cslc = slice(c0, c0 + cw_)
        lo = c0 - rc * CW
        pST = ps_st.tile([P, CW], f32, tag="pst")
        nc.tensor.matmul(pST[:, lo:lo + cw_], KTs[:, kslc],
                         QTs[h][:, cslc], start=True, stop=False)
        nc.tensor.matmul(pST[:, lo:lo + cw_], KTaug[:, kslc],
                         QTaug[h][:, cslc], start=False, stop=True)
        nc.scalar.activation(PTt[kc][:, lo:lo + cw_],
                             pST[:, lo:lo + cw_], Exp)
        if c0 == kc * P:
            # zero P above the diagonal: keep row >= key
            nc.gpsimd.affine_select(
                out=PTt[kc][:, lo:lo + P], in_=PTt[kc][:, lo:lo + P],
                compare_op=AOp.is_ge, fill=0.0,
                base=0, pattern=[[1, P]], channel_multiplier=-1,
            )

    # transposed PV: psO rows = [PV | denom]
    psO = ps_st.tile([65, CW], f32, tag="pso", bufs=1)
    for kc in range((rc + 1) * 4):
        c0 = max(kc * P, rc * CW)
        cw_ = (rc + 1) * CW - c0
        lo = c0 - rc * CW
        nc.tensor.matmul(
            psO[:, lo:lo + cw_], Vsb[kc][:, 0:65], PTt[kc][:, lo:lo + cw_],
            start=(kc == 0), stop=(kc == (rc + 1) * 4 - 1),
        )
    # normalize columns by 1/denom via gpsimd broadcast
    rd32 = ph3.tile([1, CW], f32, tag="rd32")
    nc.vector.reciprocal(rd32[:], psO[64:65, :])
    dbcS = ph3.tile([HD, CW], f32, tag="dbcs")
    nc.gpsimd.partition_broadcast(dbcS[:], rd32[:], channels=HD)
    ft, sub = h // 2, h % 2
    nc.vector.tensor_mul(aTh[ft][sub * HD:(sub + 1) * HD, rs],
                         psO[0:HD, :], dbcS[:])


def _ternary(w):
    th = np.abs(w).mean(dtype=np.float64)
    return (np.sign(w) * (np.abs(w) > th)).astype(np.float32)


_CACHE = {}


def kernel(x, q_w, q_g, k_w, k_g, v_w, o_w, o_g, qk_gain):
    x = np.asarray(x, np.float32)
    wq_eff = (_ternary(np.asarray(q_w)) * np.asarray(q_g)[None, :]
              * np.float32(qk_gain) / np.float32(np.sqrt(np.float32(HD))))
    wk_eff = _ternary(np.asarray(k_w)) * np.asarray(k_g)[None, :]
    wo_eff = _ternary(np.asarray(o_w)) * np.asarray(o_g)[None, :]
    wqT = np.ascontiguousarray(wq_eff.T).astype(np.float16)      # [D, H*HD]
    wkT = wk_eff.T.astype(np.float16)                            # [D, HKV*HD]
    wvT = np.asarray(v_w, np.float32).T.astype(np.float16)
    woT = np.ascontiguousarray(wo_eff.T).astype(np.float16)      # [D, D]

    # per-token rms scales (host); x_hat = x * r, V un-normalized on device
    # by std = 1/r
    xs = x.astype(np.float64)
    ssq = (xs * xs).mean(-1) + np.float64(EPS)
    r = (1.0 / np.sqrt(ssq)).astype(np.float32)                  # [B, T]
    std = np.sqrt(ssq).astype(np.float32)                        # [B, T]
    xn = (x * r[:, :, None]).astype(np.float32)

    if "nc" not in _CACHE:
        _CACHE["nc"] = _build()
    nc = _CACHE["nc"]

    in_maps = []
    for core in range(8):
        b, g = divmod(core, 4)
        xnT = np.ascontiguousarray(xn[b].T)                      # [D, T] f32
        xh = xnT.astype(np.float16)
        wkv_c = np.concatenate(
            [wkT[:, g * HD:(g + 1) * HD], wvT[:, g * HD:(g + 1) * HD]], axis=1)
        im = {
            "xh": xh,
            "stdc": np.ascontiguousarray(std[b].reshape(KB, P).T),
            "wq": np.ascontiguousarray(wqT[:, g * LF:(g + 1) * LF]),
            "wkv": np.ascontiguousarray(wkv_c),
            "wo": np.ascontiguousarray(woT[g * LF:(g + 1) * LF, :]),
        }
        if X_PASSES == 2:
            im["xl"] = (xnT - xh.astype(np.float32)).astype(np.float16)
        in_maps.append(im)
    _CACHE["in_maps"] = in_maps
    res = run_bass_kernel_spmd(nc, in_maps, list(range(8)))

    out = np.empty((B, T, D), np.float32)
    for b in range(B):
        ssq_a = np.zeros((T,), np.float32)
        ysum = np.zeros((T, D), np.float32)
        for g in range(4):
            rr = res.results[b * 4 + g]
            ysum += rr["yp"]
            ssq_a += rr["ssqa"][0]
        ro = 1.0 / np.sqrt(ssq_a / np.float32(D) + EPS)
        out[b] = ysum * ro[:, None]
    return out


if __name__ == "__main__":
    data = np.load("/root/problem/inputs.npz")
    out = kernel(**{k: data[k] for k in data.files})
    ref = np.load("/root/problem/ref_out.npy")
    d = out.astype(np.float64) - ref.astype(np.float64)
    rv = (d * d).sum() / (ref.astype(np.float64) ** 2).sum()
    print("resid_var=%.3e relerr=%.3e absmax=%.3g" %
          (rv, np.sqrt(rv), np.abs(d).max()))


# revision 43
# speedup vs baseline: 1.0858x; 1.0002x over previous
"""TRN2 Bass kernel for nn_AttentionCell (BitLinear GQA attention cell).

Sharding (8 cores): data-parallel over batch (2) x tensor-parallel over the
4 KV head-groups (4 query heads each). Each core computes Q/K/V projections,
causal softmax attention for its 4 heads, and a row-parallel partial of the
output projection; the host sums the 4 partials per batch and applies the
final RMSNorm row scale.

Precision: x streams as an fp16 hi/lo pair (~22-bit); Q and K are fp16
pairs ([ql;qh]/[kh;kl]); scores use a 2-pass matmul (one 128-row cross
pass, one 65-row aug pass [kh;ones] x [qh;-rowmax]). The ternary weights
are exactly representable in fp16. The row max comes from a separate fp16
stats pass in [row,key] layout (masked diag block, DVE row-reduce),
negated+transposed back via a padded matmul against -I (columns at
stride 32 so the rows land on legal partition offsets). exp on ACT to
fp16 P; causal zeroing of the diagonal P block runs on GPSIMD post-exp;
V is single fp16 in [token,feat] layout with a ones column that yields
softmax denominators inside the PV matmul; normalization multiplies by a
GPSIMD partition-broadcast of the reciprocal denominator row.

Scheduling: the row-max stats for row-chunk 0 are software-pipelined into
the projection phase; in the attention phase a flat (row-chunk, head)
pipeline overlaps each step's PE work with the stats reduces of the step
three ahead (emission order: augrow(i+1), attn(i), stats(i+3)), keeping
DVE (reduces) and PE (matmuls) concurrently busy.
"""

import numpy as np
import ml_dtypes

import concourse.bass as bass
import concourse.bacc as bacc
import concourse.mybir as mybir
import concourse.tile as tile
from concourse.bass_utils import run_bass_kernel_spmd
from concourse.masks import make_identity

f32 = mybir.dt.float32
f16 = mybir.dt.float16

EPS = np.float32(1.1920929e-07)
B, T, D = 2, 2048, 1024
H, HKV, HD = 16, 4, 64
NH = 4            # local (per-core) query heads
LF = NH * HD      # 256 local q features
P = 128
DT = D // P       # 8 d-tiles
CH = 4            # 512-wide token chunks
CW = 512
QB = T // P       # 16 query row blocks
KB = T // P       # 16 key blocks
NEG = -1.0e30

Exp = mybir.ActivationFunctionType.Exp
AOp = mybir.AluOpType
AxX = mybir.AxisListType.X

# 1 = x in single fp16 (q/k pairs absorb projection rounding only);
# 2 = x as fp16 hi/lo pair (22-bit x, bulletproof precision)
X_PASSES = 2


def _build():
    nc = bacc.Bacc("TRN2", target_bir_lowering=False, debug=False)

    xh_d = nc.dram_tensor("xh", [D, T], f16, kind="ExternalInput").ap()
    if X_PASSES == 2:
        xl_d = nc.dram_tensor("xl", [D, T], f16, kind="ExternalInput").ap()
    std_d = nc.dram_tensor("stdc", [P, KB], f32, kind="ExternalInput").ap()
    wq = nc.dram_tensor("wq", [D, LF], f16, kind="ExternalInput").ap()
    wkv = nc.dram_tensor("wkv", [D, P], f16, kind="ExternalInput").ap()
    wo = nc.dram_tensor("wo", [LF, D], f16, kind="ExternalInput").ap()
    yp = nc.dram_tensor("yp", [T, D], f32, kind="ExternalOutput").ap()
    ssqa = nc.dram_tensor("ssqa", [1, T], f32, kind="ExternalOutput").ap()

    with tile.TileContext(nc) as tc:
        with (
            tc.tile_pool(name="const", bufs=1) as const,
            tc.tile_pool(name="persist", bufs=1) as persist,
        ):
            ident16 = const.tile([P, P], f16, tag="ident16")
            make_identity(nc, ident16[:])
            negident = const.tile([P, P], f16, tag="negident")
            make_identity(nc, negident[:])
            nc.gpsimd.tensor_scalar_mul(negident[:], negident[:], -1.0)
            # stats mask, positioned in the LAST 128 cols of a 512 window
            # (S layout [row, key]): key > row -> NEG
            mask512 = const.tile([P, CW], f32, tag="mask512")
            nc.gpsimd.memset(mask512[:], 0.0)
            nc.gpsimd.affine_select(
                out=mask512[:, CW - P:CW], in_=mask512[:, CW - P:CW],
                compare_op=AOp.is_ge, fill=NEG,
                base=0, pattern=[[-1, P]], channel_multiplier=1,
            )

            stdc = persist.tile([P, KB], f32, tag="stdc")
            nc.sync.dma_start(out=stdc[:], in_=std_d[:])
            wq_sb = persist.tile([P, DT, LF], f16, tag="wq_sb")
            nc.sync.dma_start(
                out=wq_sb[:], in_=wq.rearrange("(dt p) f -> p dt f", p=P)
            )
            wkv_sb = persist.tile([P, DT, P], f16, tag="wkv_sb")
            nc.sync.dma_start(
                out=wkv_sb[:], in_=wkv.rearrange("(dt p) f -> p dt f", p=P)
            )
            wo_sb = persist.tile([P, 2, D], f16, tag="wo_sb")
            nc.sync.dma_start(
                out=wo_sb[:], in_=wo.rearrange("(ft p) o -> p ft o", p=P)
            )

            # QTaug = [qh; -max], QTs = [ql; qh]; KTaug = [kh; ones],
            # KTs = [kh; kl].  The lo parts absorb the psum->fp16 rounding
            # (and, with X_PASSES == 2, the x rounding as well).
            QTaug = [persist.tile([65, T], f16, tag=f"qta{h}", name=f"qta{h}")
                     for h in range(NH)]
            QTs = [persist.tile([P, T], f16, tag=f"qts{h}", name=f"qts{h}")
                   for h in range(NH)]
            KTaug = persist.tile([65, T], f16, tag="kta")
            KTs = persist.tile([P, T], f16, tag="kts")
            nc.gpsimd.memset(KTaug[64:65, :], 1.0)
            # V in transposed [token, feat] layout, fp16, col 64 = ones
            Vsb = [persist.tile([P, 65], f16, tag=f"v{kb}", name=f"v{kb}")
                   for kb in range(KB)]
            for kb in range(KB):
                nc.gpsimd.memset(Vsb[kb][:, 64:65], 1.0)
            aTh = [persist.tile([P, T], f16, tag=f"ath{ft}", name=f"ath{ft}")
                   for ft in range(2)]
            onesc = persist.tile([P, 1], f16, tag="onesc")
            nc.gpsimd.memset(onesc[:], 1.0)
            ssqrow = persist.tile([1, T], f32, tag="ssqrow")

            # --- stats helpers: software-pipelined masked row maxes ---
            # mallA holds per-head max columns at stride 32 so the
            # transposed rows land on legal partition offsets (0/32/64/96)
            def emit_stats(mstat, ps_stat, rc, h, mallA_h):
                nc.gpsimd.memset(mallA_h[:], 0.0)
                for j in range(4):
                    qb = rc * 4 + j
                    nk = (qb + 1) * P
                    nchunks = (nk + CW - 1) // CW
                    mxs = mstat.tile([P, 4], f32, tag="mxs")
                    for ic in range(nchunks):
                        k0 = ic * CW
                        kw = min(CW, nk - k0)
                        last = ic == nchunks - 1
                        pS = ps_stat.tile([P, CW], f32, tag="pstat")
                        nc.tensor.matmul(
                            pS[:, :kw],
                            QTaug[h][0:HD, qb * P:(qb + 1) * P],
                            KTaug[0:HD, k0:k0 + kw],
                            start=True, stop=True,
                        )
                        if last:
                            # diag-block mask add, then row max
                            nc.vector.tensor_add(
                                pS[:, kw - P:kw], pS[:, kw - P:kw],
                                mask512[:, CW - P:CW])
                            dst = (mallA_h[:, 32 * j:32 * j + 1]
                                   if nchunks == 1 else mxs[:, ic:ic + 1])
                            nc.vector.reduce_max(dst, pS[:, :kw], axis=AxX)
                        else:
                            nc.vector.reduce_max(mxs[:, ic:ic + 1],
                                                 pS[:, :kw], axis=AxX)
                    if nchunks > 1:
                        nc.vector.reduce_max(mallA_h[:, 32 * j:32 * j + 1],
                                             mxs[:, 0:nchunks], axis=AxX)

            def emit_augrow(ps_nm, rc, h, mallA_h):
                # negated transpose of the row maxes for one head
                pnm = ps_nm.tile([97, P], f32, tag="pnm",
                                 name=f"pnm_{rc}_{h}")
                nc.tensor.matmul(pnm[:], mallA_h[:], negident[:],
                                 start=True, stop=True)
                for j in range(4):
                    qb = rc * 4 + j
                    row = pnm[32 * j:32 * j + 1, :]
                    dst = QTaug[h][HD:HD + 1, qb * P:(qb + 1) * P]
                    nc.scalar.copy(dst, row)

            with tc.tile_pool(name="mstat", bufs=2) as mstat:
                mallA = {}

                def alloc_mallA(rc, h):
                    mallA[(rc, h)] = mstat.tile(
                        [P, 97], f16, tag=f"malla{h}",
                        name=f"malla_{rc}_{h}", bufs=3)
                    return mallA[(rc, h)]

                # -------------- phase 2: projections + stats(0) --------------
                with (
                    tc.tile_pool(name="ph2", bufs=8) as ph2,
                    tc.tile_pool(name="ph2v", bufs=2) as ph2v,
                    tc.tile_pool(name="ps_q", bufs=3, space="PSUM") as ps_q,
                    tc.tile_pool(name="ps_kv", bufs=2, space="PSUM") as ps_kv,
                    tc.tile_pool(name="ps_vt", bufs=1, space="PSUM") as ps_vt,
                    tc.tile_pool(name="ps_st0", bufs=2, space="PSUM") as ps_st0,
                ):
                    for c in range(CH):
                        cs = slice(c * CW, (c + 1) * CW)
                        psQ = [ps_q.tile([P, CW], f32, tag="psq",
                                         name=f"psq_{c}_{i}") for i in range(2)]
                        psKV = ps_kv.tile([P, CW], f32, tag="pskv")
                        for d in range(DT):
                            th = ph2.tile([P, CW], f16, tag="th")
                            nc.sync.dma_start(out=th[:],
                                              in_=xh_d[d * P:(d + 1) * P, cs])
                            xparts = [th]
                            if X_PASSES == 2:
                                tl = ph2.tile([P, CW], f16, tag="tl")
                                nc.sync.dma_start(
                                    out=tl[:], in_=xl_d[d * P:(d + 1) * P, cs])
                                xparts.append(tl)
                            first, last = d == 0, d == DT - 1
                            for ft in range(2):
                                for xi, tx in enumerate(xparts):
                                    nc.tensor.matmul(
                                        psQ[ft][:],
                                        wq_sb[:, d, ft * P:(ft + 1) * P], tx[:],
                                        start=first and xi == 0,
                                        stop=last and xi == len(xparts) - 1)
                            for xi, tx in enumerate(xparts):
                                nc.tensor.matmul(
                                    psKV[:], wkv_sb[:, d, :], tx[:],
                                    start=first and xi == 0,
                                    stop=last and xi == len(xparts) - 1)
                        for ft in range(2):
                            for sub in range(2):
                                h = 2 * ft + sub
                                pslc = psQ[ft][sub * HD:(sub + 1) * HD, :]
                                nc.scalar.copy(QTaug[h][0:HD, cs], pslc)
                                nc.gpsimd.tensor_copy(QTs[h][HD:P, cs],
                                                      QTaug[h][0:HD, cs])
                                nc.vector.tensor_sub(QTs[h][0:HD, cs], pslc,
                                                     QTaug[h][0:HD, cs])
                        nc.scalar.copy(KTaug[0:HD, cs], psKV[0:HD, :])
                        nc.gpsimd.tensor_copy(KTs[0:HD, cs], KTaug[0:HD, cs])
                        nc.vector.tensor_sub(KTs[HD:P, cs], psKV[0:HD, :],
                                             KTaug[0:HD, cs])
                        # V (rows 64:128 of psKV) -> fp16, transpose to
                        # [token, feat], un-normalize by per-token std
                        vt = ph2v.tile([HD, CW], f16, tag="vt")
                        nc.scalar.copy(vt[:], psKV[HD:P, :])
                        for s4 in range(4):
                            kb = c * 4 + s4
                            pvt = ps_vt.tile([P, HD], f16, tag="pvt")
                            nc.tensor.transpose(pvt[:],
                                                vt[:, s4 * P:(s4 + 1) * P],
                                                ident16[0:HD, 0:HD])
                            nc.scalar.mul(Vsb[kb][:, 0:HD], pvt[:],
                                          stdc[:, kb:kb + 1])
                        # stats(rc=0, head c) rides along with chunk c
                        emit_stats(mstat, ps_st0, 0, c, alloc_mallA(0, c))

                # ------ phases 3-6: attention + O-proj, per row-chunk ------
                with (
                    tc.tile_pool(name="ph3", bufs=6) as ph3,
                    tc.tile_pool(name="ptp", bufs=1) as ptp,
                    tc.tile_pool(name="ph6", bufs=4) as ph6,
                    tc.tile_pool(name="ps_stat", bufs=2, space="PSUM") as ps_stat,
                    tc.tile_pool(name="ps_st", bufs=2, space="PSUM") as ps_st,
                    tc.tile_pool(name="ps_nm", bufs=1, space="PSUM") as ps_nm,
                    tc.tile_pool(name="ps_y", bufs=2, space="PSUM") as ps_y,
                ):
                    PTt = [ptp.tile([P, CW], f16, tag=f"pt{kc}", name=f"pt{kc}")
                           for kc in range(KB)]
                    # flat (rc, h) pipeline with one-step stats lookahead:
                    # attn(step i) overlaps stats+augrow of step i+1, whose
                    # DVE size tracks step i's PE size
                    steps = [(rc, h) for rc in range(4) for h in range(NH)]
                    emit_augrow(ps_nm, 0, 0, mallA[(0, 0)])
                    for i, (rc, h) in enumerate(steps):
                        rs = slice(rc * CW, (rc + 1) * CW)
                        # augrow for the NEXT step first so its small copies
                        # sit early in the DVE/ACT queues
                        if i + 1 < len(steps):
                            rcn, hn = steps[i + 1]
                            emit_augrow(ps_nm, rcn, hn, mallA[(rcn, hn)])
                        attention_head(nc, tc, ph3, ptp, ph6, ps_st, ps_y,
                                       PTt, QTaug, QTs, KTaug, KTs, Vsb,
                                       aTh, rc, h, rs)
                        # three-step stats lookahead
                        if i == 0:
                            for ii in (2, 3):
                                rc2, h2 = steps[ii]
                                if rc2 >= 1:
                                    emit_stats(mstat, ps_stat, rc2, h2,
                                               alloc_mallA(rc2, h2))
                        elif i + 3 < len(steps):
                            rc2, h2 = steps[i + 3]
                            if rc2 >= 1:
                                emit_stats(mstat, ps_stat, rc2, h2,
                                           alloc_mallA(rc2, h2))
                        if h == NH - 1:
                            # ssq of the normalized attn rows + O-projection
                            psq = ps_y.tile([1, CW], f32, tag="psy",
                                            name=f"psq_{rc}")
                            for ft in range(2):
                                sqt = ph6.tile([P, CW], f16, tag="sqt")
                                nc.gpsimd.tensor_mul(sqt[:], aTh[ft][:, rs],
                                                     aTh[ft][:, rs])
                                nc.tensor.matmul(psq[:], onesc[:], sqt[:],
                                                 start=(ft == 0),
                                                 stop=(ft == 1))
                            nc.vector.tensor_copy(ssqrow[0:1, rs], psq[:])
                            for j in range(4):
                                qb = rc * 4 + j
                                qs = slice(qb * P, (qb + 1) * P)
                                for oc in range(2):
                                    os_ = slice(oc * CW, (oc + 1) * CW)
                                    psY = ps_y.tile([P, CW], f32, tag="psy",
                                                    name=f"psY_{qb}_{oc}")
                                    for ft in range(2):
                                        nc.tensor.matmul(
                                            psY[:], aTh[ft][:, qs],
                                            wo_sb[:, ft, os_],
                                            start=(ft == 0), stop=(ft == 1))
                                    ysb = ph6.tile([P, CW], f32, tag="ysb")
                                    nc.scalar.copy(ysb[:], psY[:])
                                    nc.sync.dma_start(out=yp[qs, os_],
                                                      in_=ysb[:])
                    nc.sync.dma_start(out=ssqa[:], in_=ssqrow[:])

    nc.finalize()
    return nc


def attention_head(nc, tc, ph3, ptp, ph6, ps_st, ps_y, PTt, QTaug, QTs,
                   KTaug, KTs, Vsb, aTh, rc, h, rs):
    # S.T for this row chunk: 2-pass (cross terms via [kh;kl]x[ql;qh],
    # hi terms + -max via the 65-row aug pass) + exp; diagonal P block
    # zeroed on GPSIMD post-exp
    for kc in range((rc + 1) * 4):
        c0 = max(kc * P, rc * CW)
        cw_ = (rc + 1) * CW - c0
        kslc = slice(kc * P, (kc + 1) * P)
        cslc = slice(c0, c0 + cw_)
        lo = c0 - rc * CW
        pST = ps_st.tile([P, CW], f32, tag="pst")
        nc.tensor.matmul(pST[:, lo:lo + cw_], KTs[:, kslc],
                         QTs[h][:, cslc], start=True, stop=False)
        nc.tensor.matmul(pST[:, lo:lo + cw_], KTaug[:, kslc],
                         QTaug[h][:, cslc], start=False, stop=True)
        nc.scalar.activation(PTt[kc][:, lo:lo + cw_],
                             pST[:, lo:lo + cw_], Exp)
        if c0 == kc * P:
            # zero P above the diagonal: keep row >= key
            nc.gpsimd.affine_select(
                out=PTt[kc][:, lo:lo + P], in_=PTt[kc][:, lo:lo + P],
                compare_op=AOp.is_ge, fill=0.0,
                base=0, pattern=[[1, P]], channel_multiplier=-1,
            )

    # transposed PV: psO rows = [PV | denom]
    psO = ps_st.tile([65, CW], f32, tag="pso", bufs=1)
    for kc in range((rc + 1) * 4):
        c0 = max(kc * P, rc * CW)
        cw_ = (rc + 1) * CW - c0
        lo = c0 - rc * CW
        nc.tensor.matmul(
            psO[:, lo:lo + cw_], Vsb[kc][:, 0:65], PTt[kc][:, lo:lo + cw_],
            start=(kc == 0), stop=(kc == (rc + 1) * 4 - 1),
        )
    # normalize columns by 1/denom via gpsimd broadcast
    rd32 = ph3.tile([1, CW], f32, tag="rd32")
    nc.vector.reciprocal(rd32[:], psO[64:65, :])
    dbcS = ph3.tile([HD, CW], f32, tag="dbcs")
    nc.gpsimd.partition_broadcast(dbcS[:], rd32[:], channels=HD)
    ft, sub = h // 2, h % 2
    nc.vector.tensor_mul(aTh[ft][sub * HD:(sub + 1) * HD, rs],
                         psO[0:HD, :], dbcS[:])


def _ternary(w):
    th = np.abs(w).mean(dtype=np.float64)
    return (np.sign(w) * (np.abs(w) > th)).astype(np.float32)


_CACHE = {}


def kernel(x, q_w, q_g, k_w, k_g, v_w, o_w, o_g, qk_gain):
    x = np.asarray(x, np.float32)
    wq_eff = (_ternary(np.asarray(q_w)) * np.asarray(q_g)[None, :]
              * np.float32(qk_gain) / np.float32(np.sqrt(np.float32(HD))))
    wk_eff = _ternary(np.asarray(k_w)) * np.asarray(k_g)[None, :]
    wo_eff = _ternary(np.asarray(o_w)) * np.asarray(o_g)[None, :]
    wqT = np.ascontiguousarray(wq_eff.T).astype(np.float16)      # [D, H*HD]
    wkT = wk_eff.T.astype(np.float16)                            # [D, HKV*HD]
    wvT = np.asarray(v_w, np.float32).T.astype(np.float16)
    woT = np.ascontiguousarray(wo_eff.T).astype(np.float16)      # [D, D]

    # per-token rms scales (host); x_hat = x * r, V un-normalized on device
    # by std = 1/r
    xs = x.astype(np.float64)
    ssq = (xs * xs).mean(-1) + np.float64(EPS)
    r = (1.0 / np.sqrt(ssq)).astype(np.float32)                  # [B, T]
    std = np.sqrt(ssq).astype(np.float32)                        # [B, T]
    xn = (x * r[:, :, None]).astype(np.float32)

    if "nc" not in _CACHE:
        _CACHE["nc"] = _build()
    nc = _CACHE["nc"]

    in_maps = []
    for core in range(8):
        b, g = divmod(core, 4)
        xnT = np.ascontiguousarray(xn[b].T)                      # [D, T] f32
        xh = xnT.astype(np.float16)
        wkv_c = np.concatenate(
            [wkT[:, g * HD:(g + 1) * HD], wvT[:, g * HD:(g + 1) * HD]], axis=1)
        im = {
            "xh": xh,
            "stdc": np.ascontiguousarray(std[b].reshape(KB, P).T),
            "wq": np.ascontiguousarray(wqT[:, g * LF:(g + 1) * LF]),
            "wkv": np.ascontiguousarray(wkv_c),
            "wo": np.ascontiguousarray(woT[g * LF:(g + 1) * LF, :]),
        }
        if X_PASSES == 2:
            im["xl"] = (xnT - xh.astype(np.float32)).astype(np.float16)
        in_maps.append(im)
    _CACHE["in_maps"] = in_maps
    res = run_bass_kernel_spmd(nc, in_maps, list(range(8)))

    out = np.empty((B, T, D), np.float32)
    for b in range(B):
        ssq_a = np.zeros((T,), np.float32)
        ysum = np.zeros((T, D), np.float32)
        for g in range(4):
            rr = res.results[b * 4 + g]
            ysum += rr["yp"]
            ssq_a += rr["ssqa"][0]
        ro = 1.0 / np.sqrt(ssq_a / np.float32(D) + EPS)
        out[b] = ysum * ro[:, None]
    return out


if __name__ == "__main__":
    data = np.load("/root/problem/inputs.npz")
    out = kernel(**{k: data[k] for k in data.files})
    ref = np.load("/root/problem/ref_out.npy")
    d = out.astype(np.float64) - ref.astype(np.float64)
    rv = (d * d).sum() / (ref.astype(np.float64) ** 2).sum()
    print("resid_var=%.3e relerr=%.3e absmax=%.3g" %
          (rv, np.sqrt(rv), np.abs(d).max()))
